# revision 15
# baseline (speedup 1.0000x reference)
"""Trainium2 Bass kernel for nn_CTMR_UncFusionNet (dense_cnn).

Sharding: 8 cores = (image b in 0..3) x (H-half). Each core computes its
64-row half on a 72-row slab (4-row halo margins, recomputed locally).
Cross-core reductions (SE stats U, 2x layernorm stats, MDTA attention
matrix A) go through 4 tiny pair-wise AllReduces.

All convs are emitted as 9-tap shifted fp32r matmuls over a flat
[C, 72*130] pitch-130 layout (pad cols emulate W zero-padding; per-core
edge masks emulate H zero-padding at image boundaries).

SBUF big-slot plan (bufs=1, manual lifetime chains):
  A [128,9360]: X -> (Xe in place) -> xn -> g (gdfn gated product)
  Bs [128,9232]: hf -> sq-scratch -> qk1 -> sq-scratch -> v1 -> xn2
  Cs [128,8844]: c1 -> x -> h0..h2 (gdfn 1x1 chunks)
  Fs [128,8592]: ls -> qk (-> qkT in place) -> v (-> attn -> x2 in place)
  Gs [128,4424]: h3 + out-chunk
"""
import sys
sys.path.insert(0, '/opt/trn_rl_repo')
import numpy as np

C = 64
B, H, W = 4, 128, 128
PITCH = W + 2            # 130
SLAB_R = 72
NFLAT = SLAB_R * PITCH   # 9360
OWN0, OWN1 = 4, 68
OWNLO, OWNHI = OWN0 * PITCH, OWN1 * PITCH      # 520, 8840
ALPHA_F = 5.0
LN_EPS = 1e-5
NPIX = float(H * W)

_CACHE = {}


# ---------------------------------------------------------------- host packing
def _pack_weights(params):
    fuse, rest = params['fuse'], params['rest']
    pm, pg = rest['mdta'], rest['gdfn']
    f32 = np.float32

    def blockdiag_tapT(w_ct, w_mr):
        out = np.zeros((2 * C, 9 * 2 * C), f32)
        for dy in range(3):
            for dx in range(3):
                t = dy * 3 + dx
                out[:C, t * 128:t * 128 + C] = np.asarray(w_ct)[:, :, dy, dx].T
                out[C:, t * 128 + C:t * 128 + 128] = np.asarray(w_mr)[:, :, dy, dx].T
        return out

    w = {}
    w['unc1_wT'] = blockdiag_tapT(fuse['unc_ct']['w1'], fuse['unc_mr']['w1'])
    w['unc1_b'] = np.concatenate([fuse['unc_ct']['b1'], fuse['unc_mr']['b1']]).astype(f32)[:, None]
    w2ct = 0.5 * np.asarray(fuse['unc_ct']['w2'])[C:]
    w2mr = 0.5 * np.asarray(fuse['unc_mr']['w2'])[C:]
    w['unc2_wT'] = blockdiag_tapT(w2ct, w2mr)
    w['unc2_b'] = (0.5 * np.concatenate([np.asarray(fuse['unc_ct']['b2'])[C:],
                                         np.asarray(fuse['unc_mr']['b2'])[C:]])).astype(f32)[:, None]
    w['fc1_wT'] = np.asarray(fuse['fc1_w']).T.astype(f32)
    w['fc1_b'] = np.asarray(fuse['fc1_b']).astype(f32)[:, None]
    w['fc2_wT'] = np.asarray(fuse['fc2_w']).T.astype(f32)
    w['fc2_b'] = np.asarray(fuse['fc2_b']).astype(f32)[:, None]
    w['mlp_wT'] = np.asarray(rest['mlp_w'])[:, :, 0, 0].T.astype(f32)
    w['mlp_b'] = np.asarray(rest['mlp_b']).astype(f32)[:, None]

    w['qk1_wT'] = np.concatenate([np.asarray(pm['q']['w1'])[:, :, 0, 0].T,
                                  np.asarray(pm['k']['w1'])[:, :, 0, 0].T], 1).astype(f32)
    w['qk1_b'] = np.concatenate([pm['q']['b1'], pm['k']['b1']]).astype(f32)[:, None]
    w['qk2_wT'] = blockdiag_tapT(pm['q']['w2'], pm['k']['w2'])
    w['qk2_b'] = np.concatenate([pm['q']['b2'], pm['k']['b2']]).astype(f32)[:, None]
    w['v1_wT'] = np.asarray(pm['v']['w1'])[:, :, 0, 0].T.astype(f32)
    w['v1_b'] = np.asarray(pm['v']['b1']).astype(f32)[:, None]
    v2 = np.asarray(pm['v']['w2'])
    w['v2_wT'] = np.concatenate([v2[:, :, dy, dx].T for dy in range(3) for dx in range(3)], 1).astype(f32)
    w['v2_b'] = np.asarray(pm['v']['b2']).astype(f32)[:, None]
    w['w4_wT'] = np.asarray(pm['w4'])[:, :, 0, 0].T.astype(f32)
    w['w4_b'] = np.asarray(pm['b4']).astype(f32)[:, None]

    d1w1 = np.asarray(pg['d1']['w1'])[:, :, 0, 0].T
    d2w1 = np.asarray(pg['d2']['w1'])[:, :, 0, 0].T
    w['dd1_wT'] = np.concatenate([d1w1, d2w1], 1).astype(f32)    # [64, 512]
    b1s = np.concatenate([pg['d1']['b1'], pg['d2']['b1']]).astype(f32)
    w['dd1_b'] = b1s.reshape(4, 128).T.copy()                    # [128, 4]
    # dd2: [mo 0..3][kb 0..1][tap 0..8] blocks [128,128] -> [128, 9216]
    dd2 = np.zeros((128, 4 * 2 * 9 * 128), f32)
    for br, pp in ((0, pg['d1']), (1, pg['d2'])):
        wv = np.asarray(pp['w2'])
        for mb in range(2):
            mo = br * 2 + mb
            for kb in range(2):
                for t in range(9):
                    dy, dx = t // 3, t % 3
                    blk = wv[mb * 128:(mb + 1) * 128, kb * 128:(kb + 1) * 128, dy, dx].T
                    off = ((mo * 2 + kb) * 9 + t) * 128
                    dd2[:, off:off + 128] = blk
    w['dd2_wT'] = dd2
    b2s = np.concatenate([pg['d1']['b2'], pg['d2']['b2']]).astype(f32)
    w['dd2_b'] = b2s.reshape(4, 128).T.copy()
    wo = np.asarray(pg['wo'])[:, :, 0, 0]
    w['wo_wT'] = np.concatenate([wo[:, :128].T, wo[:, 128:].T], 1).astype(f32)
    w['wo_b'] = np.asarray(pg['bo']).astype(f32)[:, None]
    w['ident'] = np.eye(128, dtype=f32)
    return w


def _make_slab(F, b, h):
    r0 = h * 64
    sl = np.zeros((C, SLAB_R, PITCH), np.float32)
    for s in range(SLAB_R):
        r = r0 - 4 + s
        rr = r if 0 <= r < H else (1 if r == -1 else (H - 2 if r == H else None))
        if rr is not None:
            sl[:, s, 1:W + 1] = F[b, :, rr, :]
            sl[:, s, 0] = F[b, :, rr, 1]
            sl[:, s, W + 1] = F[b, :, rr, W - 2]
    return sl.reshape(C, NFLAT)


def _make_edge(h):
    m = np.zeros((SLAB_R, PITCH), np.float32)
    r0 = h * 64
    for s in list(range(0, 4)) + list(range(68, 72)):
        if 0 <= r0 - 4 + s < H:
            m[s, :] = 1.0
    flat = m.reshape(-1)
    return np.concatenate([flat[0:520], flat[8840:9360]])[None, :].copy()


# ---------------------------------------------------------------- bass builder
def _build():
    import concourse.bacc as bacc
    import concourse.tile as tile
    from concourse import mybir
    from concourse.alu_op_type import AluOpType as ALU

    AF = mybir.ActivationFunctionType
    F32 = mybir.dt.float32
    F32R = mybir.dt.float32r
    AX = mybir.AxisListType

    nc = bacc.Bacc("TRN2", target_bir_lowering=False, debug=False, num_devices=8)

    R_PARAMS = {'xct', 'xmr', 'unc1_wT', 'unc2_wT', 'qk2_wT', 'v2_wT', 'dd2_wT',
                'mlp_wT', 'qk1_wT', 'v1_wT', 'w4_wT', 'dd1_wT', 'wo_wT'}
    ins = {}
    for name, shape in [('xct', [C, NFLAT]), ('xmr', [C, NFLAT]), ('edge', [1, 1040])]:
        dt_ = F32R if name in R_PARAMS else F32
        ins[name] = nc.declare_dram_parameter(name, shape, dt_, isOutput=False)
    wshapes = {
        'unc1_wT': [128, 1152], 'unc1_b': [128, 1], 'unc2_wT': [128, 1152], 'unc2_b': [128, 1],
        'fc1_wT': [128, 8], 'fc1_b': [8, 1], 'fc2_wT': [8, 128], 'fc2_b': [128, 1],
        'mlp_wT': [128, 64], 'mlp_b': [64, 1],
        'qk1_wT': [64, 128], 'qk1_b': [128, 1], 'qk2_wT': [128, 1152], 'qk2_b': [128, 1],
        'v1_wT': [64, 64], 'v1_b': [64, 1], 'v2_wT': [64, 576], 'v2_b': [64, 1],
        'w4_wT': [64, 64], 'w4_b': [64, 1],
        'dd1_wT': [64, 512], 'dd1_b': [128, 4], 'dd2_wT': [128, 9216], 'dd2_b': [128, 4],
        'wo_wT': [128, 128], 'wo_b': [64, 1], 'ident': [128, 128],
    }
    for name, shape in wshapes.items():
        dt_ = F32R if name in R_PARAMS else F32
        ins[name] = nc.declare_dram_parameter(name, shape, dt_, isOutput=False)
    out_t = nc.declare_dram_parameter('out', [C, 64 * W], F32, isOutput=True)

    cc = {}
    for nm, shp in [('u', [128, 1]), ('l1', [64, 2]), ('a', [64, 64]), ('l2', [64, 2])]:
        cc[nm + '_out'] = nc.dram_tensor(f'cc_{nm}_out', shp, F32)
    RG = [[0, 1], [2, 3], [4, 5], [6, 7]]

    def allreduce(nm):
        nc.gpsimd.collective_compute('AllReduce', mybir.AluOpType.add,
                                     replica_groups=RG,
                                     ins=[cc[nm + '_in'].opt()], outs=[cc[nm + '_out'][:]])

    def ntiles(lo, hi, step=512):
        r, p = [], lo
        while p < hi:
            n = min(step, hi - p)
            r.append((p, n))
            p += n
        return r

    with tile.TileContext(nc) as tc:
        import contextlib
        est = contextlib.ExitStack()
        with est:
            wpool = est.enter_context(tc.tile_pool(name="wts", bufs=1))
            big = est.enter_context(tc.tile_pool(name="big", bufs=1))
            psum = est.enter_context(tc.tile_pool(name="psum", bufs=3, space="PSUM"))
            psum64 = est.enter_context(tc.tile_pool(name="psum64", bufs=2, space="PSUM"))
            psA = est.enter_context(tc.tile_pool(name="psA", bufs=1, space="PSUM"))
            pstr = est.enter_context(tc.tile_pool(name="pstr", bufs=1, space="PSUM"))
            small = est.enter_context(tc.tile_pool(name="small", bufs=1))
            drp = est.enter_context(tc.tile_pool(name="drp", bufs=1, space="DRAM"))
            cc['u_in'] = drp.tile([128, 1], F32, tag="ccu", name="ccu_in")
            cc['l1_in'] = drp.tile([64, 2], F32, tag="ccl1", name="ccl1_in")
            cc['a_in'] = drp.tile([64, 64], F32, tag="cca", name="cca_in")
            cc['l2_in'] = drp.tile([64, 2], F32, tag="ccl2", name="ccl2_in")

            # ----- persistent small weights (fp32r via ACT copy; biases stay f32)
            wt = {}
            SMALL_W = ['mlp_wT', 'qk1_wT', 'v1_wT', 'w4_wT', 'dd1_wT', 'wo_wT']
            BIASES = ['unc1_b', 'unc2_b', 'fc1_b', 'fc2_b', 'mlp_b', 'qk1_b', 'qk2_b',
                      'v1_b', 'v2_b', 'w4_b', 'dd1_b', 'dd2_b', 'wo_b']
            for name in BIASES + ['ident', 'fc1_wT', 'fc2_wT']:
                t = wpool.tile(wshapes[name], F32, tag=name, name=name)
                nc.sync.dma_start(t[:], ins[name][:])
                wt[name] = t
            for name in SMALL_W:
                t = wpool.tile(wshapes[name], F32R, tag=name, name=name)
                nc.sync.dma_start(t[:], ins[name][:])
                nc.scalar.activation(t[:], t[:].bitcast(F32), AF.Copy)
                wt[name] = t
            idr = wpool.tile([128, 128], F32R, tag="identr")
            nc.vector.tensor_copy(idr[:], wt['ident'][:])

            # two alternating fp32r slots for the big 3x3 weights (in-place round)
            _w3n = [0]

            def load_w3x3(name):
                t = wpool.tile([128, 1152], F32R, tag=f"w3r{_w3n[0] % 2}", name=f"w3r_{name}")
                _w3n[0] += 1
                p, f = wshapes[name]
                sub = t[0:p, 0:f]
                nc.sync.dma_start(sub, ins[name][:])
                nc.scalar.activation(sub, sub.bitcast(F32), AF.Copy)
                return t

            dd2r = [wpool.tile([128, 1152], F32R, tag=f"dd2r{i}", name=f"dd2r{i}") for i in range(2)]

            zerot = wpool.tile([128, 160], F32, tag="zerot")
            nc.gpsimd.memset(zerot[:], 0.0)
            zr = zerot[:].bitcast(F32R)

            edge1 = wpool.tile([1, 1040], F32, tag="edge1")
            nc.sync.dma_start(edge1[:], ins['edge'][:])
            edge = wpool.tile([128, 1040], F32, tag="edge")
            nc.gpsimd.partition_broadcast(edge[:], edge1[:])

            # ----- big slots
            A_ = big.tile([128, NFLAT], F32, tag="A")
            Bs = big.tile([128, 9232], F32, tag="B")
            Cs = big.tile([128, 8844], F32, tag="C")
            Fs = big.tile([128, 8592], F32, tag="F")
            Gs = big.tile([128, 4424], F32, tag="G")

            X = A_
            nc.sync.dma_start(X[0:64, :].bitcast(F32R), ins['xct'][:])
            nc.sync.dma_start(X[64:128, :].bitcast(F32R), ins['xmr'][:])

            def padcol_memset(bufap, row_lo, row_hi, base=0, pdim=128):
                # zero cols 0,129 of rows [row_lo,row_hi), F32R-typed writes
                R = row_hi - row_lo
                ap = bufap[0:pdim, base:base + R * PITCH] \
                    .bitcast(F32R).rearrange("p (r c) -> p r c", c=PITCH)
                zin = zr[0:pdim, 0:R].rearrange("p (r c) -> p r c", c=1)
                nc.vector.tensor_copy(ap[:, :, 0:1], zin)
                nc.vector.tensor_copy(ap[:, :, 129:130], zin)

            def guard_memset(bufap, col, pdim=128):
                nc.vector.tensor_copy(bufap[0:pdim, col:col + 1].bitcast(F32R),
                                      zr[0:pdim, 0:1])

            def band_mul(bufap, blo, bhi, flat_lo, pdim=128):
                # in-place multiply by edge mask; F32R-typed out
                if bhi <= blo:
                    return
                fl = flat_lo
                ecol = fl if fl < 520 else 520 + (fl - 8840)
                nc.vector.tensor_tensor(bufap[0:pdim, blo:bhi].bitcast(F32R),
                                        bufap[0:pdim, blo:bhi],
                                        edge[0:pdim, ecol:ecol + (bhi - blo)], op=ALU.mult)

            # =================== Phase 1: hf + unc chain ===================
            # hf in Bs: col j <-> flat j (use [131, 9229))
            hf = Bs
            hfr = Bs[:].bitcast(F32R)
            lo, hi = 131, 9229
            nc.vector.tensor_scalar(hfr[:, lo:hi], X[:, lo:hi], -4.0, None, ALU.mult)
            for off in (-PITCH, PITCH, -1, 1):
                nc.vector.tensor_tensor(hfr[:, lo:hi], hf[:, lo:hi],
                                        X[:, lo + off:hi + off], op=ALU.add)
            band_mul(hf, 131, 520, 131)
            band_mul(hf, 8840, 9229, 8840)
            padcol_memset(hf, 1, 71, base=PITCH)

            # unc1: relu(conv+b) -> c1 in Cs (col j <-> flat j+389, range [390,8970))
            CS_OFF = 389
            c1r = Cs[:].bitcast(F32R)
            wu1 = load_w3x3('unc1_wT')
            for tlo, tn in ntiles(390, 8970):
                ps = psum.tile([128, 512], F32, tag="cps")
                for t in range(9):
                    off = (t // 3 - 1) * PITCH + (t % 3 - 1)
                    nc.tensor.matmul(ps[:, 0:tn], wu1[:, t * 128:(t + 1) * 128],
                                     hfr[:, tlo + off:tlo + off + tn],
                                     start=(t == 0), stop=(t == 8))
                nc.scalar.activation(c1r[:, tlo - CS_OFF:tlo - CS_OFF + tn], ps[:, 0:tn],
                                     AF.Relu, bias=wt['unc1_b'][:])
            band_mul(Cs, 390 - CS_OFF, 520 - CS_OFF, 390)
            band_mul(Cs, 8840 - CS_OFF, 8970 - CS_OFF, 8840)
            padcol_memset(Cs, 3, 69, base=390 - CS_OFF)
            guard_memset(Cs, 0)
            guard_memset(Cs, 8970 - CS_OFF)

            # unc2 over owned [520,8840): ls in Fs (col j <-> flat j+520)
            FS_OFF = 520
            ls = Fs
            wu2 = load_w3x3('unc2_wT')
            for tlo, tn in ntiles(OWNLO, OWNHI):
                ps = psum.tile([128, 512], F32, tag="cps")
                for t in range(9):
                    off = (t // 3 - 1) * PITCH + (t % 3 - 1)
                    nc.tensor.matmul(ps[:, 0:tn], wu2[:, t * 128:(t + 1) * 128],
                                     c1r[:, tlo + off - CS_OFF:tlo + off - CS_OFF + tn],
                                     start=(t == 0), stop=(t == 8))
                d = ls[:, tlo - FS_OFF:tlo - FS_OFF + tn]
                nc.vector.tensor_scalar(d.bitcast(F32R), ps[:, 0:tn],
                                        wt['unc2_b'][:], None, ALU.add)
                nc.vector.tensor_scalar(d.bitcast(F32R), d, -5.0, 5.0, ALU.max, ALU.min)

            # U partial: in-place exp over ls (owned rows, image cols) + accum
            u_part = small.tile([128, 1], F32, tag="u_part")
            ls3 = ls[:, 0:8320].rearrange("p (r c) -> p r c", c=PITCH)[:, :, 1:129]
            ls3r = ls[:, 0:8320].bitcast(F32R).rearrange("p (r c) -> p r c", c=PITCH)[:, :, 1:129]
            nc.scalar.activation(ls3r, ls3, AF.Exp, accum_out=u_part[:])

            nc.sync.dma_start(cc['u_in'][:], u_part[:])
            allreduce('u')
            u_full = small.tile([128, 1], F32, tag="u_full")
            nc.sync.dma_start(u_full[:], cc['u_out'][:])

            # ---- SE head
            ones = small.tile([128, 1], F32, tag="ones")
            nc.gpsimd.memset(ones[:], 1.0)
            psS = pstr.tile([1, 1], F32, tag="tiny", name="psS")
            nc.tensor.matmul(psS[:], u_full[:], ones[:], start=True, stop=True)
            s_sc = small.tile([1, 1], F32, tag="s_sc")
            nc.vector.tensor_scalar(s_sc[:], psS[:], float(1.0 / (128 * NPIX)), 1e-6,
                                    ALU.mult, ALU.add)
            nc.vector.reciprocal(s_sc[:], s_sc[:])
            d_b = small.tile([128, 1], F32, tag="d_b")
            nc.gpsimd.partition_broadcast(d_b[:], s_sc[:])
            scl = small.tile([128, 1], F32, tag="scl")
            nc.vector.tensor_scalar(scl[:], d_b[:], float(-ALPHA_F / NPIX), None, ALU.mult)
            Rv = small.tile([128, 1], F32, tag="Rv")
            nc.scalar.activation(Rv[:], u_full[:], AF.Exp, scale=scl[:])
            psZ = pstr.tile([8, 1], F32, tag="tiny", name="psZ")
            nc.tensor.matmul(psZ[:], wt['fc1_wT'][:], Rv[:], start=True, stop=True)
            zt = small.tile([8, 1], F32, tag="zt")
            nc.scalar.activation(zt[:], psZ[:], AF.Relu, bias=wt['fc1_b'][:])
            psW = pstr.tile([128, 1], F32, tag="tiny", name="psW")
            nc.tensor.matmul(psW[:], wt['fc2_wT'][:], zt[:], start=True, stop=True)
            wch1 = small.tile([128, 1], F32, tag="wch1")
            nc.scalar.activation(wch1[:], psW[:], AF.Sigmoid, bias=wt['fc2_b'][:])
            nc.vector.tensor_scalar(wch1[:], wch1[:], 1.0, None, ALU.add)

            # Xe = X*(1+wch) in place (fp32r view), rows [2,70)
            Xe = A_[:].bitcast(F32R)
            nc.vector.scalar_tensor_tensor(Xe[:, 260:9100], A_[:, 260:9100],
                                           wch1[:], A_[:, 260:9100],
                                           op0=ALU.mult, op1=ALU.bypass)

            # =================== Phase 2: mlp + LN1 ===================
            # x in Cs (col j <-> flat j+CS_OFF... x range [260,9100): use CX_OFF=259)
            CX_OFF = 259
            x = Cs
            for tlo, tn in ntiles(260, 9100):
                ps = psum64.tile([64, 512], F32, tag="c64")
                nc.tensor.matmul(ps[0:64, 0:tn], wt['mlp_wT'][:], Xe[:, tlo:tlo + tn],
                                 start=True, stop=True)
                nc.vector.tensor_scalar(
                    x[0:64, tlo - CX_OFF:tlo - CX_OFF + tn].bitcast(F32R),
                    ps[0:64, 0:tn], wt['mlp_b'][:], None, ALU.add)

            ln1 = small.tile([64, 2], F32, tag="ln1")
            x3 = x[0:64, OWNLO - CX_OFF:OWNHI - CX_OFF] \
                .rearrange("p (r c) -> p r c", c=PITCH)[:, :, 1:129]
            nc.vector.reduce_sum(ln1[:, 0:1], x3, axis=AX.XY)
            sq1 = Bs[0:64, 0:8320].bitcast(F32R) \
                .rearrange("p (r c) -> p r c", c=PITCH)[:, :, 1:129]
            nc.scalar.activation(sq1, x3, AF.Square, accum_out=ln1[:, 1:2])
            nc.sync.dma_start(cc['l1_in'][:], ln1[:])
            allreduce('l1')
            ln1f = small.tile([64, 2], F32, tag="ln1f")
            nc.sync.dma_start(ln1f[:], cc['l1_out'][:])

            def ln_coeffs(lnf, tag):
                mu = small.tile([64, 1], F32, tag=tag + "mu")
                nc.vector.tensor_scalar(mu[:], lnf[:, 0:1], float(1.0 / NPIX), None, ALU.mult)
                musq = small.tile([64, 1], F32, tag=tag + "ms")
                nc.vector.tensor_tensor(musq[:], mu[:], mu[:], op=ALU.mult)
                var = small.tile([64, 1], F32, tag=tag + "va")
                nc.vector.scalar_tensor_tensor(var[:], lnf[:, 1:2], float(1.0 / NPIX),
                                               musq[:], op0=ALU.mult, op1=ALU.subtract)
                nc.vector.tensor_scalar(var[:], var[:], LN_EPS, None, ALU.add)
                sd = small.tile([64, 1], F32, tag=tag + "sd")
                nc.scalar.activation(sd[:], var[:], AF.Sqrt)
                inv = small.tile([64, 1], F32, tag=tag + "iv")
                nc.vector.reciprocal(inv[:], sd[:])
                nb = small.tile([64, 1], F32, tag=tag + "nb")
                nc.vector.tensor_tensor(nb[:], mu[:], inv[:], op=ALU.mult)
                nc.vector.tensor_scalar(nb[:], nb[:], -1.0, None, ALU.mult)
                return inv, nb

            inv1, nb1 = ln_coeffs(ln1f, "l1")
            # xn in A_ (in place over X/Xe; col j <-> flat j), fp32r, rows [2,70)
            xn = A_[:].bitcast(F32R)
            nc.scalar.activation(xn[0:64, 260:9100], x[0:64, 260 - CX_OFF:9100 - CX_OFF],
                                 AF.Identity, scale=inv1[:], bias=nb1[:])
            band_mul(A_, 260, 520, 260, pdim=64)
            band_mul(A_, 8840, 9100, 8840, pdim=64)
            padcol_memset(A_, 2, 70, base=260, pdim=64)

            # =================== Phase 3: MDTA ===================
            # qk1 in Bs (col j <-> flat j, range [390, 8970))
            qk1r = Bs[:].bitcast(F32R)
            for tlo, tn in ntiles(390, 8970):
                ps = psum.tile([128, 512], F32, tag="cps")
                nc.tensor.matmul(ps[:, 0:tn], wt['qk1_wT'][:], xn[0:64, tlo:tlo + tn],
                                 start=True, stop=True)
                nc.vector.tensor_scalar(qk1r[:, tlo:tlo + tn], ps[:, 0:tn],
                                        wt['qk1_b'][:], None, ALU.add)
            band_mul(Bs, 390, 520, 390)
            band_mul(Bs, 8840, 8970, 8840)
            padcol_memset(Bs, 3, 69, base=390)
            guard_memset(Bs, 389)
            guard_memset(Bs, 8970)

            # qk conv3x3 -> Fs (col j <-> flat j+FS_OFF=520), fp32r, padcols zeroed
            qkr = Fs[:].bitcast(F32R)
            wq2 = load_w3x3('qk2_wT')
            for tlo, tn in ntiles(OWNLO, OWNHI):
                ps = psum.tile([128, 512], F32, tag="cps")
                for t in range(9):
                    off = (t // 3 - 1) * PITCH + (t % 3 - 1)
                    nc.tensor.matmul(ps[:, 0:tn], wq2[:, t * 128:(t + 1) * 128],
                                     qk1r[:, tlo + off:tlo + off + tn],
                                     start=(t == 0), stop=(t == 8))
                nc.vector.tensor_scalar(qkr[:, tlo - FS_OFF:tlo - FS_OFF + tn],
                                        ps[:, 0:tn], wt['qk2_b'][:], None, ALU.add)
            padcol_memset(Fs, 4, 68, base=0)

            # transpose 65 blocks in place (qkT block i overwrites qk block i)
            for i in range(65):
                pst_t = pstr.tile([128, 128], F32R, tag="trps")
                nc.tensor.transpose(pst_t[:], qkr[:, i * 128:(i + 1) * 128], idr[:])
                nc.vector.tensor_copy(Fs[:, i * 128:(i + 1) * 128].bitcast(F32R), pst_t[:])
            psa = psA.tile([64, 64], F32, tag="Aps")
            for i in range(65):
                nc.tensor.matmul(psa[:], qkr[:, i * 128 + 64:(i + 1) * 128],
                                 qkr[:, i * 128:i * 128 + 64],
                                 start=(i == 0), stop=(i == 64))
            a_part = small.tile([64, 64], F32, tag="a_part")
            nc.vector.tensor_copy(a_part[:], psa[:])
            nc.sync.dma_start(cc['a_in'][:], a_part[:])
            allreduce('a')
            A_full = small.tile([64, 64], F32, tag="A_full")
            nc.sync.dma_start(A_full[:], cc['a_out'][:])

            # softmax over 4096 entries
            rmax = small.tile([64, 1], F32, tag="rmax")
            nc.vector.reduce_max(rmax[:], A_full[:], axis=AX.X)
            pt1 = pstr.tile([1, 64], F32, tag="tiny", name="pt1")
            nc.tensor.transpose(pt1[:], rmax[:], wt['ident'][0:64, 0:64])
            rmT = small.tile([1, 64], F32, tag="rmT")
            nc.vector.tensor_copy(rmT[:], pt1[:])
            gmax = small.tile([1, 1], F32, tag="gmax")
            nc.vector.reduce_max(gmax[:], rmT[:], axis=AX.X)
            nc.vector.tensor_scalar(gmax[:], gmax[:], -1.0, None, ALU.mult)
            ngmax = small.tile([64, 1], F32, tag="ngmax")
            nc.gpsimd.partition_broadcast(ngmax[:], gmax[:])
            E = small.tile([64, 64], F32, tag="E")
            rsum = small.tile([64, 1], F32, tag="rsum")
            nc.scalar.activation(E[:], A_full[:], AF.Exp, bias=ngmax[:], accum_out=rsum[:])
            pt2 = pstr.tile([1, 64], F32, tag="tiny", name="pt2")
            nc.tensor.transpose(pt2[:], rsum[:], wt['ident'][0:64, 0:64])
            rsT = small.tile([1, 64], F32, tag="rsT")
            nc.vector.tensor_copy(rsT[:], pt2[:])
            gsum = small.tile([1, 1], F32, tag="gsum")
            nc.vector.reduce_sum(gsum[:], rsT[:], axis=AX.X)
            nc.vector.reciprocal(gsum[:], gsum[:])
            isum = small.tile([64, 1], F32, tag="isum")
            nc.gpsimd.partition_broadcast(isum[:], gsum[:])
            A_sm = small.tile([64, 64], F32R, tag="A_sm")
            nc.scalar.activation(A_sm[:], E[:], AF.Identity, scale=isum[:])

            # v chain: v1 in Bs (col j <-> flat j, [260,9100)); v in Fs ([390,8970),
            # col j <-> flat j+FV_OFF=389)
            v1r = Bs[:].bitcast(F32R)
            for tlo, tn in ntiles(260, 9100):
                ps = psum64.tile([64, 512], F32, tag="c64")
                nc.tensor.matmul(ps[0:64, 0:tn], wt['v1_wT'][:], xn[0:64, tlo:tlo + tn],
                                 start=True, stop=True)
                nc.vector.tensor_scalar(v1r[0:64, tlo:tlo + tn], ps[0:64, 0:tn],
                                        wt['v1_b'][:], None, ALU.add)
            band_mul(Bs, 260, 520, 260, pdim=64)
            band_mul(Bs, 8840, 9100, 8840, pdim=64)
            padcol_memset(Bs, 2, 70, base=260, pdim=64)
            guard_memset(Bs, 259)
            guard_memset(Bs, 9100)

            FV_OFF = 389
            vr = Fs[:].bitcast(F32R)
            wv2 = load_w3x3('v2_wT')
            for tlo, tn in ntiles(390, 8970):
                ps = psum64.tile([64, 512], F32, tag="c64")
                for t in range(9):
                    off = (t // 3 - 1) * PITCH + (t % 3 - 1)
                    nc.tensor.matmul(ps[0:64, 0:tn], wv2[0:64, t * 64:(t + 1) * 64],
                                     v1r[0:64, tlo + off:tlo + off + tn],
                                     start=(t == 0), stop=(t == 8))
                nc.vector.tensor_scalar(vr[0:64, tlo - FV_OFF:tlo - FV_OFF + tn],
                                        ps[0:64, 0:tn], wt['v2_b'][:], None, ALU.add)

            # attn = A_sm^T v (in place over v), then x2 = w4 attn + b4 + x (in place)
            for tlo, tn in ntiles(390, 8970):
                ps = psum64.tile([64, 512], F32, tag="c64")
                nc.tensor.matmul(ps[0:64, 0:tn], A_sm[:],
                                 vr[0:64, tlo - FV_OFF:tlo - FV_OFF + tn],
                                 start=True, stop=True)
                nc.vector.tensor_copy(vr[0:64, tlo - FV_OFF:tlo - FV_OFF + tn],
                                      ps[0:64, 0:tn])
            x2 = Fs
            for tlo, tn in ntiles(390, 8970):
                ps = psum64.tile([64, 512], F32, tag="c64")
                nc.tensor.matmul(ps[0:64, 0:tn], wt['w4_wT'][:],
                                 vr[0:64, tlo - FV_OFF:tlo - FV_OFF + tn],
                                 start=True, stop=True)
                nc.vector.scalar_tensor_tensor(
                    x2[0:64, tlo - FV_OFF:tlo - FV_OFF + tn].bitcast(F32R),
                    ps[0:64, 0:tn], wt['w4_b'][:],
                    x[0:64, tlo - CX_OFF:tlo - CX_OFF + tn],
                    op0=ALU.add, op1=ALU.add)

            # LN2
            ln2 = small.tile([64, 2], F32, tag="ln2")
            x23 = x2[0:64, OWNLO - FV_OFF:OWNHI - FV_OFF] \
                .rearrange("p (r c) -> p r c", c=PITCH)[:, :, 1:129]
            nc.vector.reduce_sum(ln2[:, 0:1], x23, axis=AX.XY)
            sq2 = Bs[0:64, 0:8320].bitcast(F32R) \
                .rearrange("p (r c) -> p r c", c=PITCH)[:, :, 1:129]
            nc.scalar.activation(sq2, x23, AF.Square, accum_out=ln2[:, 1:2])
            nc.sync.dma_start(cc['l2_in'][:], ln2[:])
            allreduce('l2')
            ln2f = small.tile([64, 2], F32, tag="ln2f")
            nc.sync.dma_start(ln2f[:], cc['l2_out'][:])
            inv2, nb2 = ln_coeffs(ln2f, "l2")

            # xn2 in Bs (col j <-> flat j, rows [3,69)), fp32r
            xn2 = Bs[:].bitcast(F32R)
            nc.scalar.activation(xn2[0:64, 390:8970],
                                 x2[0:64, 390 - FV_OFF:8970 - FV_OFF],
                                 AF.Identity, scale=inv2[:], bias=nb2[:])
            band_mul(Bs, 390, 520, 390, pdim=64)
            band_mul(Bs, 8840, 8970, 8840, pdim=64)
            padcol_memset(Bs, 3, 69, base=390, pdim=64)

            # =================== Phase 4: GDFN (4 chunks of 16 rows) ===================
            HSZ = 18 * PITCH  # 2340
            OSZ = 16 * PITCH  # 2080
            # h tiles: h0..h2 in Cs at [0,2342),[2342,4684),[4684,7026); h3 in Gs [0,2342)
            # g tiles: g_lo in A_[:, 0:2080], g_hi in A_[:, 2080:4160]
            # out tile: Gs[64, 2342:4422]
            for ci in range(4):
                r_out0 = 4 + 16 * ci
                hbase = (r_out0 - 1) * PITCH
                hs = []
                for mo in range(4):
                    if mo < 3:
                        ht = Cs[:, mo * 2342:(mo + 1) * 2342]
                    else:
                        ht = Gs[:, 0:2342]
                    htr = ht.bitcast(F32R)
                    for tlo, tn in ntiles(0, HSZ):
                        ps = psum.tile([128, 512], F32, tag="cps")
                        nc.tensor.matmul(ps[:, 0:tn],
                                         wt['dd1_wT'][:, mo * 128:(mo + 1) * 128],
                                         xn2[0:64, hbase + tlo:hbase + tlo + tn],
                                         start=True, stop=True)
                        nc.vector.tensor_scalar(htr[:, 1 + tlo:1 + tlo + tn],
                                                ps[:, 0:tn],
                                                wt['dd1_b'][:, mo:mo + 1], None, ALU.add)
                    if ci == 0:
                        nc.vector.tensor_tensor(htr[:, 1:131], ht[:, 1:131],
                                                edge[:, 390:520], op=ALU.mult)
                    if ci == 3:
                        nc.vector.tensor_tensor(htr[:, 1 + 17 * PITCH:1 + HSZ],
                                                ht[:, 1 + 17 * PITCH:1 + HSZ],
                                                edge[:, 520:650], op=ALU.mult)
                    hap = htr[:, 1:1 + HSZ].rearrange("p (r c) -> p r c", c=PITCH)
                    zin18 = zr[:, 0:18].rearrange("p (r c) -> p r c", c=1)
                    nc.vector.tensor_copy(hap[:, :, 0:1], zin18)
                    nc.vector.tensor_copy(hap[:, :, 129:130], zin18)
                    nc.vector.tensor_copy(htr[:, 0:1], zr[:, 0:1])
                    nc.vector.tensor_copy(htr[:, 1 + HSZ:2 + HSZ], zr[:, 0:1])
                    hs.append(htr)

                # conv3x3 per (mo): d1 -> gelu into g tiles; d2 -> g = gelu*d2 (in place)
                for mo in range(4):
                    br, mb = mo // 2, mo % 2
                    gdst = A_[:, mb * 2080:(mb + 1) * 2080]
                    for kb in range(2):
                        t = dd2r[kb]
                        nc.sync.dma_start(
                            t[:],
                            ins['dd2_wT'][:, ((mo * 2 + kb) * 9) * 128:
                                          ((mo * 2 + kb) * 9 + 9) * 128])
                        nc.scalar.activation(t[:], t[:].bitcast(F32), AF.Copy)
                    for tlo, tn in ntiles(0, OSZ):
                        ps = psum.tile([128, 512], F32, tag="cps")
                        first = True
                        for kb in range(2):
                            src = hs[br * 2 + kb]
                            for t in range(9):
                                off = (t // 3 - 1) * PITCH + (t % 3 - 1)
                                base = 131 + tlo + off
                                nc.tensor.matmul(ps[:, 0:tn],
                                                 dd2r[kb][:, t * 128:(t + 1) * 128],
                                                 src[:, base:base + tn],
                                                 start=first, stop=(kb == 1 and t == 8))
                                first = False
                        if br == 0:
                            nc.scalar.activation(gdst[:, tlo:tlo + tn].bitcast(F32R),
                                                 ps[:, 0:tn],
                                                 AF.Gelu, bias=wt['dd2_b'][:, mo:mo + 1])
                        else:
                            nc.vector.scalar_tensor_tensor(
                                gdst[:, tlo:tlo + tn].bitcast(F32R), ps[:, 0:tn],
                                wt['dd2_b'][:, mo:mo + 1], gdst[:, tlo:tlo + tn],
                                op0=ALU.add, op1=ALU.mult)

                # wo + residual + DMA out
                outt = Gs[0:64, 2342:4422]
                obase = r_out0 * PITCH
                gr = A_[:].bitcast(F32R)
                for tlo, tn in ntiles(0, OSZ):
                    ps = psum64.tile([64, 512], F32, tag="c64")
                    for kb in range(2):
                        nc.tensor.matmul(ps[0:64, 0:tn],
                                         wt['wo_wT'][:, kb * 64:(kb + 1) * 64],
                                         gr[:, kb * 2080 + tlo:kb * 2080 + tlo + tn],
                                         start=(kb == 0), stop=(kb == 1))
                    nc.vector.scalar_tensor_tensor(
                        outt[:, tlo:tlo + tn].bitcast(F32R), ps[0:64, 0:tn],
                        wt['wo_b'][:],
                        x2[0:64, obase + tlo - FV_OFF:obase + tlo - FV_OFF + tn],
                        op0=ALU.add, op1=ALU.add)
                oap = outt[:, :].rearrange("p (r c) -> p r c", c=PITCH)[:, :, 1:129]
                nc.sync.dma_start(
                    out_t[:].rearrange("p (r c) -> p r c", c=W)[:, 16 * ci:16 * ci + 16, :],
                    oap)

    nc.compile()
    return nc


def _get_nc():
    if 'nc' not in _CACHE:
        _CACHE['nc'] = _build()
    return _CACHE['nc']


def make_in_maps(F_ct, F_mr, params):
    F_ct = np.asarray(F_ct, np.float32)
    F_mr = np.asarray(F_mr, np.float32)
    w = _pack_weights(params)
    in_maps = []
    for c in range(8):
        b, h = c // 2, c % 2
        m = dict(w)
        m['xct'] = _make_slab(F_ct, b, h)
        m['xmr'] = _make_slab(F_mr, b, h)
        m['edge'] = _make_edge(h)
        in_maps.append(m)
    return in_maps


def run(F_ct, F_mr, params, trace=False):
    from concourse.bass_utils import run_bass_kernel_spmd
    nc = _get_nc()
    in_maps = make_in_maps(F_ct, F_mr, params)
    kw = {}
    if trace:
        kw = dict(trace=True)
    res = run_bass_kernel_spmd(nc, in_maps, list(range(8)), **kw)
    out = np.zeros((B, C, H, W), np.float32)
    for c in range(8):
        b, h = c // 2, c % 2
        out[b, :, h * 64:(h + 1) * 64, :] = res.results[c]['out'].reshape(C, 64, W)
    return out, res


def kernel(F_ct, F_mr, params):
    out, _ = run(F_ct, F_mr, params)
    return out


# revision 17
# speedup vs baseline: 1.2789x; 1.2789x over previous
"""Trainium2 Bass kernel for nn_CTMR_UncFusionNet (dense_cnn).

Sharding: 8 cores = (image b in 0..3) x (H-half). Each core computes its
64-row half on a 72-row slab (4-row halo margins, recomputed locally).
Cross-core reductions (SE stats U, 2x layernorm stats, MDTA attention
matrix A) go through 4 tiny pair-wise AllReduces.

All convs are emitted as 9-tap shifted fp32r matmuls over a flat
[C, 72*130] pitch-130 layout (pad cols emulate W zero-padding; per-core
edge masks emulate H zero-padding at image boundaries).

SBUF big-slot plan (bufs=1, manual lifetime chains):
  A [128,9360]: X -> (Xe in place) -> xn -> g (gdfn gated product)
  Bs [128,9232]: hf -> sq-scratch -> qk1 -> sq-scratch -> v1 -> xn2
  Cs [128,8844]: c1 -> x -> h0..h2 (gdfn 1x1 chunks)
  Fs [128,8592]: ls -> qk (-> qkT in place) -> v (-> attn -> x2 in place)
  Gs [128,4424]: h3 + out-chunk
"""
import sys
sys.path.insert(0, '/opt/trn_rl_repo')
import numpy as np

C = 64
B, H, W = 4, 128, 128
PITCH = W + 2            # 130
SLAB_R = 72
NFLAT = SLAB_R * PITCH   # 9360
OWN0, OWN1 = 4, 68
OWNLO, OWNHI = OWN0 * PITCH, OWN1 * PITCH      # 520, 8840
ALPHA_F = 5.0
LN_EPS = 1e-5
NPIX = float(H * W)

_CACHE = {}


# ---------------------------------------------------------------- host packing
def _pack_weights(params):
    fuse, rest = params['fuse'], params['rest']
    pm, pg = rest['mdta'], rest['gdfn']
    f32 = np.float32

    def blockdiag_tapT(w_ct, w_mr):
        out = np.zeros((2 * C, 9 * 2 * C), f32)
        for dy in range(3):
            for dx in range(3):
                t = dy * 3 + dx
                out[:C, t * 128:t * 128 + C] = np.asarray(w_ct)[:, :, dy, dx].T
                out[C:, t * 128 + C:t * 128 + 128] = np.asarray(w_mr)[:, :, dy, dx].T
        return out

    w = {}
    w['unc1_wT'] = blockdiag_tapT(fuse['unc_ct']['w1'], fuse['unc_mr']['w1'])
    w['unc1_b'] = np.concatenate([fuse['unc_ct']['b1'], fuse['unc_mr']['b1']]).astype(f32)[:, None]
    w2ct = 0.5 * np.asarray(fuse['unc_ct']['w2'])[C:]
    w2mr = 0.5 * np.asarray(fuse['unc_mr']['w2'])[C:]
    w['unc2_wT'] = blockdiag_tapT(w2ct, w2mr)
    w['unc2_b'] = (0.5 * np.concatenate([np.asarray(fuse['unc_ct']['b2'])[C:],
                                         np.asarray(fuse['unc_mr']['b2'])[C:]])).astype(f32)[:, None]
    w['fc1_wT'] = np.asarray(fuse['fc1_w']).T.astype(f32)
    w['fc1_b'] = np.asarray(fuse['fc1_b']).astype(f32)[:, None]
    w['fc2_wT'] = np.asarray(fuse['fc2_w']).T.astype(f32)
    w['fc2_b'] = np.asarray(fuse['fc2_b']).astype(f32)[:, None]
    w['mlp_wT'] = np.asarray(rest['mlp_w'])[:, :, 0, 0].T.astype(f32)
    w['mlp_b'] = np.asarray(rest['mlp_b']).astype(f32)[:, None]

    w['qk1_wT'] = np.concatenate([np.asarray(pm['q']['w1'])[:, :, 0, 0].T,
                                  np.asarray(pm['k']['w1'])[:, :, 0, 0].T], 1).astype(f32)
    w['qk1_b'] = np.concatenate([pm['q']['b1'], pm['k']['b1']]).astype(f32)[:, None]
    w['qk2_wT'] = blockdiag_tapT(pm['q']['w2'], pm['k']['w2'])
    w['qk2_b'] = np.concatenate([pm['q']['b2'], pm['k']['b2']]).astype(f32)[:, None]
    w['v1_wT'] = np.asarray(pm['v']['w1'])[:, :, 0, 0].T.astype(f32)
    w['v1_b'] = np.asarray(pm['v']['b1']).astype(f32)[:, None]
    v2 = np.asarray(pm['v']['w2'])
    w['v2_wT'] = np.concatenate([v2[:, :, dy, dx].T for dy in range(3) for dx in range(3)], 1).astype(f32)
    w['v2_b'] = np.asarray(pm['v']['b2']).astype(f32)[:, None]
    w['w4_wT'] = np.asarray(pm['w4'])[:, :, 0, 0].T.astype(f32)
    w['w4_b'] = np.asarray(pm['b4']).astype(f32)[:, None]

    d1w1 = np.asarray(pg['d1']['w1'])[:, :, 0, 0].T
    d2w1 = np.asarray(pg['d2']['w1'])[:, :, 0, 0].T
    w['dd1_wT'] = np.concatenate([d1w1, d2w1], 1).astype(f32)    # [64, 512]
    b1s = np.concatenate([pg['d1']['b1'], pg['d2']['b1']]).astype(f32)
    w['dd1_b'] = b1s.reshape(4, 128).T.copy()                    # [128, 4]
    # dd2: [mo 0..3][kb 0..1][tap 0..8] blocks [128,128] -> [128, 9216]
    dd2 = np.zeros((128, 4 * 2 * 9 * 128), f32)
    for br, pp in ((0, pg['d1']), (1, pg['d2'])):
        wv = np.asarray(pp['w2'])
        for mb in range(2):
            mo = br * 2 + mb
            for kb in range(2):
                for t in range(9):
                    dy, dx = t // 3, t % 3
                    blk = wv[mb * 128:(mb + 1) * 128, kb * 128:(kb + 1) * 128, dy, dx].T
                    off = ((mo * 2 + kb) * 9 + t) * 128
                    dd2[:, off:off + 128] = blk
    w['dd2_wT'] = dd2
    b2s = np.concatenate([pg['d1']['b2'], pg['d2']['b2']]).astype(f32)
    w['dd2_b'] = b2s.reshape(4, 128).T.copy()
    wo = np.asarray(pg['wo'])[:, :, 0, 0]
    w['wo_wT'] = np.concatenate([wo[:, :128].T, wo[:, 128:].T], 1).astype(f32)
    w['wo_b'] = np.asarray(pg['bo']).astype(f32)[:, None]
    w['ident'] = np.eye(128, dtype=f32)
    return w


def _make_slab(F, b, h):
    r0 = h * 64
    sl = np.zeros((C, SLAB_R, PITCH), np.float32)
    for s in range(SLAB_R):
        r = r0 - 4 + s
        rr = r if 0 <= r < H else (1 if r == -1 else (H - 2 if r == H else None))
        if rr is not None:
            sl[:, s, 1:W + 1] = F[b, :, rr, :]
            sl[:, s, 0] = F[b, :, rr, 1]
            sl[:, s, W + 1] = F[b, :, rr, W - 2]
    return sl.reshape(C, NFLAT)


def _make_edge(h):
    m = np.zeros((SLAB_R, PITCH), np.float32)
    r0 = h * 64
    for s in list(range(0, 4)) + list(range(68, 72)):
        if 0 <= r0 - 4 + s < H:
            m[s, :] = 1.0
    flat = m.reshape(-1)
    return np.concatenate([flat[0:520], flat[8840:9360]])[None, :].copy()


# ---------------------------------------------------------------- bass builder
def _build():
    import concourse.bacc as bacc
    import concourse.tile as tile
    from concourse import mybir
    from concourse.alu_op_type import AluOpType as ALU

    AF = mybir.ActivationFunctionType
    F32 = mybir.dt.float32
    F32R = mybir.dt.float32r
    AX = mybir.AxisListType

    nc = bacc.Bacc("TRN2", target_bir_lowering=False, debug=False, num_devices=8)

    R_PARAMS = {'xct', 'xmr', 'unc1_wT', 'unc2_wT', 'qk2_wT', 'v2_wT', 'dd2_wT',
                'mlp_wT', 'qk1_wT', 'v1_wT', 'w4_wT', 'dd1_wT', 'wo_wT'}
    ins = {}
    for name, shape in [('xct', [C, NFLAT]), ('xmr', [C, NFLAT]), ('edge', [1, 1040])]:
        dt_ = F32R if name in R_PARAMS else F32
        ins[name] = nc.declare_dram_parameter(name, shape, dt_, isOutput=False)
    wshapes = {
        'unc1_wT': [128, 1152], 'unc1_b': [128, 1], 'unc2_wT': [128, 1152], 'unc2_b': [128, 1],
        'fc1_wT': [128, 8], 'fc1_b': [8, 1], 'fc2_wT': [8, 128], 'fc2_b': [128, 1],
        'mlp_wT': [128, 64], 'mlp_b': [64, 1],
        'qk1_wT': [64, 128], 'qk1_b': [128, 1], 'qk2_wT': [128, 1152], 'qk2_b': [128, 1],
        'v1_wT': [64, 64], 'v1_b': [64, 1], 'v2_wT': [64, 576], 'v2_b': [64, 1],
        'w4_wT': [64, 64], 'w4_b': [64, 1],
        'dd1_wT': [64, 512], 'dd1_b': [128, 4], 'dd2_wT': [128, 9216], 'dd2_b': [128, 4],
        'wo_wT': [128, 128], 'wo_b': [64, 1], 'ident': [128, 128],
    }
    for name, shape in wshapes.items():
        dt_ = F32R if name in R_PARAMS else F32
        ins[name] = nc.declare_dram_parameter(name, shape, dt_, isOutput=False)
    out_t = nc.declare_dram_parameter('out', [C, 64 * W], F32, isOutput=True)

    cc = {}
    for nm, shp in [('u', [128, 1]), ('l1', [64, 2]), ('a', [64, 64]), ('l2', [64, 2])]:
        cc[nm + '_out'] = nc.dram_tensor(f'cc_{nm}_out', shp, F32)
    RG = [[0, 1], [2, 3], [4, 5], [6, 7]]

    def allreduce(nm):
        nc.gpsimd.collective_compute('AllReduce', mybir.AluOpType.add,
                                     replica_groups=RG,
                                     ins=[cc[nm + '_in'].opt()], outs=[cc[nm + '_out'][:]])

    def ntiles(lo, hi, step=512):
        r, p = [], lo
        while p < hi:
            n = min(step, hi - p)
            r.append((p, n))
            p += n
        return r

    with tile.TileContext(nc) as tc:
        import contextlib
        est = contextlib.ExitStack()
        with est:
            wpool = est.enter_context(tc.tile_pool(name="wts", bufs=1))
            big = est.enter_context(tc.tile_pool(name="big", bufs=1))
            psum = est.enter_context(tc.tile_pool(name="psum", bufs=3, space="PSUM"))
            psum64 = est.enter_context(tc.tile_pool(name="psum64", bufs=2, space="PSUM"))
            psA = est.enter_context(tc.tile_pool(name="psA", bufs=1, space="PSUM"))
            pstr = est.enter_context(tc.tile_pool(name="pstr", bufs=1, space="PSUM"))
            small = est.enter_context(tc.tile_pool(name="small", bufs=1))
            drp = est.enter_context(tc.tile_pool(name="drp", bufs=1, space="DRAM"))
            cc['u_in'] = drp.tile([128, 1], F32, tag="ccu", name="ccu_in")
            cc['l1_in'] = drp.tile([64, 2], F32, tag="ccl1", name="ccl1_in")
            cc['a_in'] = drp.tile([64, 64], F32, tag="cca", name="cca_in")
            cc['l2_in'] = drp.tile([64, 2], F32, tag="ccl2", name="ccl2_in")

            # ----- persistent small weights (fp32r via ACT copy; biases stay f32)
            wt = {}
            SMALL_W = ['mlp_wT', 'qk1_wT', 'v1_wT', 'w4_wT', 'dd1_wT', 'wo_wT']
            BIASES = ['unc1_b', 'unc2_b', 'fc1_b', 'fc2_b', 'mlp_b', 'qk1_b', 'qk2_b',
                      'v1_b', 'v2_b', 'w4_b', 'dd1_b', 'dd2_b', 'wo_b']
            for name in BIASES + ['ident', 'fc1_wT', 'fc2_wT']:
                t = wpool.tile(wshapes[name], F32, tag=name, name=name)
                nc.sync.dma_start(t[:], ins[name][:])
                wt[name] = t
            for name in SMALL_W:
                t = wpool.tile(wshapes[name], F32R, tag=name, name=name)
                nc.sync.dma_start(t[:], ins[name][:])
                nc.scalar.activation(t[:], t[:].bitcast(F32), AF.Copy)
                wt[name] = t
            idr = wpool.tile([128, 128], F32R, tag="identr")
            nc.vector.tensor_copy(idr[:], wt['ident'][:])

            # two alternating fp32r slots for the big 3x3 weights (in-place round)
            _w3n = [0]

            def load_w3x3(name):
                t = wpool.tile([128, 1152], F32R, tag=f"w3r{_w3n[0] % 2}", name=f"w3r_{name}")
                _w3n[0] += 1
                p, f = wshapes[name]
                sub = t[0:p, 0:f]
                nc.sync.dma_start(sub, ins[name][:])
                nc.scalar.activation(sub, sub.bitcast(F32), AF.Copy)
                return t

            dd2r = [wpool.tile([128, 1152], F32R, tag=f"dd2r{i}", name=f"dd2r{i}") for i in range(2)]

            zerot = wpool.tile([128, 160], F32, tag="zerot")
            nc.gpsimd.memset(zerot[:], 0.0)
            zr = zerot[:].bitcast(F32R)

            edge1 = wpool.tile([1, 1040], F32, tag="edge1")
            nc.sync.dma_start(edge1[:], ins['edge'][:])
            edge = wpool.tile([128, 1040], F32, tag="edge")
            nc.gpsimd.partition_broadcast(edge[:], edge1[:])

            # ----- big slots
            A_ = big.tile([128, NFLAT], F32, tag="A")
            Bs = big.tile([128, 9232], F32, tag="B")
            Cs = big.tile([128, 8844], F32, tag="C")
            Fs = big.tile([128, 8592], F32, tag="F")
            Gs = big.tile([128, 4424], F32, tag="G")

            X = A_
            nc.sync.dma_start(X[0:64, :].bitcast(F32R), ins['xct'][:])
            nc.sync.dma_start(X[64:128, :].bitcast(F32R), ins['xmr'][:])

            def padcol_memset(bufap, row_lo, row_hi, base=0, pdim=128):
                # zero cols 0,129 of rows [row_lo,row_hi), F32R-typed writes
                R = row_hi - row_lo
                ap = bufap[0:pdim, base:base + R * PITCH] \
                    .bitcast(F32R).rearrange("p (r c) -> p r c", c=PITCH)
                zin = zr[0:pdim, 0:R].rearrange("p (r c) -> p r c", c=1)
                nc.vector.tensor_copy(ap[:, :, 0:1], zin)
                nc.vector.tensor_copy(ap[:, :, 129:130], zin)

            def guard_memset(bufap, col, pdim=128):
                nc.vector.tensor_copy(bufap[0:pdim, col:col + 1].bitcast(F32R),
                                      zr[0:pdim, 0:1])

            def band_mul(bufap, blo, bhi, flat_lo, pdim=128):
                # in-place multiply by edge mask; F32R-typed out
                if bhi <= blo:
                    return
                fl = flat_lo
                ecol = fl if fl < 520 else 520 + (fl - 8840)
                nc.vector.tensor_tensor(bufap[0:pdim, blo:bhi].bitcast(F32R),
                                        bufap[0:pdim, blo:bhi],
                                        edge[0:pdim, ecol:ecol + (bhi - blo)], op=ALU.mult)

            # =================== Phase 1: hf + unc chain ===================
            # hf in Bs: col j <-> flat j (use [131, 9229))
            hf = Bs
            hfr = Bs[:].bitcast(F32R)
            lo, hi = 131, 9229
            nc.vector.tensor_scalar(hfr[:, lo:hi], X[:, lo:hi], -4.0, None, ALU.mult)
            for off in (-PITCH, PITCH, -1, 1):
                nc.vector.tensor_tensor(hfr[:, lo:hi], hf[:, lo:hi],
                                        X[:, lo + off:hi + off], op=ALU.add)
            band_mul(hf, 131, 520, 131)
            band_mul(hf, 8840, 9229, 8840)
            padcol_memset(hf, 1, 71, base=PITCH)

            # unc1: relu(conv+b) -> c1 in Cs (col j <-> flat j+389, range [390,8970))
            CS_OFF = 389
            c1r = Cs[:].bitcast(F32R)
            wu1 = load_w3x3('unc1_wT')
            for tlo, tn in ntiles(390, 8970):
                ps = psum.tile([128, 512], F32, tag="cps")
                for t in range(9):
                    off = (t // 3 - 1) * PITCH + (t % 3 - 1)
                    nc.tensor.matmul(ps[:, 0:tn], wu1[:, t * 128:(t + 1) * 128],
                                     hfr[:, tlo + off:tlo + off + tn],
                                     start=(t == 0), stop=(t == 8))
                nc.scalar.activation(c1r[:, tlo - CS_OFF:tlo - CS_OFF + tn], ps[:, 0:tn],
                                     AF.Relu, bias=wt['unc1_b'][:])
            band_mul(Cs, 390 - CS_OFF, 520 - CS_OFF, 390)
            band_mul(Cs, 8840 - CS_OFF, 8970 - CS_OFF, 8840)
            padcol_memset(Cs, 3, 69, base=390 - CS_OFF)
            guard_memset(Cs, 0)
            guard_memset(Cs, 8970 - CS_OFF)

            # unc2 over owned [520,8840): ls in Fs (col j <-> flat j+520)
            FS_OFF = 520
            ls = Fs
            wu2 = load_w3x3('unc2_wT')
            for tlo, tn in ntiles(OWNLO, OWNHI):
                ps = psum.tile([128, 512], F32, tag="cps")
                for t in range(9):
                    off = (t // 3 - 1) * PITCH + (t % 3 - 1)
                    nc.tensor.matmul(ps[:, 0:tn], wu2[:, t * 128:(t + 1) * 128],
                                     c1r[:, tlo + off - CS_OFF:tlo + off - CS_OFF + tn],
                                     start=(t == 0), stop=(t == 8))
                d = ls[:, tlo - FS_OFF:tlo - FS_OFF + tn]
                nc.vector.tensor_scalar(d.bitcast(F32R), ps[:, 0:tn],
                                        wt['unc2_b'][:], None, ALU.add)
                nc.vector.tensor_scalar(d.bitcast(F32R), d, -5.0, 5.0, ALU.max, ALU.min)

            # U partial: in-place exp over ls (owned rows, image cols) + accum
            u_part = small.tile([128, 1], F32, tag="u_part")
            ls3 = ls[:, 0:8320].rearrange("p (r c) -> p r c", c=PITCH)[:, :, 1:129]
            ls3r = ls[:, 0:8320].bitcast(F32R).rearrange("p (r c) -> p r c", c=PITCH)[:, :, 1:129]
            nc.scalar.activation(ls3r, ls3, AF.Exp, accum_out=u_part[:])

            nc.sync.dma_start(cc['u_in'][:], u_part[:])
            allreduce('u')
            u_full = small.tile([128, 1], F32, tag="u_full")
            nc.sync.dma_start(u_full[:], cc['u_out'][:])

            # ---- SE head
            ones = small.tile([128, 1], F32, tag="ones")
            nc.gpsimd.memset(ones[:], 1.0)
            psS = pstr.tile([1, 1], F32, tag="tiny", name="psS")
            nc.tensor.matmul(psS[:], u_full[:], ones[:], start=True, stop=True)
            s_sc = small.tile([1, 1], F32, tag="s_sc")
            nc.vector.tensor_scalar(s_sc[:], psS[:], float(1.0 / (128 * NPIX)), 1e-6,
                                    ALU.mult, ALU.add)
            nc.vector.reciprocal(s_sc[:], s_sc[:])
            d_b = small.tile([128, 1], F32, tag="d_b")
            nc.gpsimd.partition_broadcast(d_b[:], s_sc[:])
            scl = small.tile([128, 1], F32, tag="scl")
            nc.vector.tensor_scalar(scl[:], d_b[:], float(-ALPHA_F / NPIX), None, ALU.mult)
            Rv = small.tile([128, 1], F32, tag="Rv")
            nc.scalar.activation(Rv[:], u_full[:], AF.Exp, scale=scl[:])
            psZ = pstr.tile([8, 1], F32, tag="tiny", name="psZ")
            nc.tensor.matmul(psZ[:], wt['fc1_wT'][:], Rv[:], start=True, stop=True)
            zt = small.tile([8, 1], F32, tag="zt")
            nc.scalar.activation(zt[:], psZ[:], AF.Relu, bias=wt['fc1_b'][:])
            psW = pstr.tile([128, 1], F32, tag="tiny", name="psW")
            nc.tensor.matmul(psW[:], wt['fc2_wT'][:], zt[:], start=True, stop=True)
            wch1 = small.tile([128, 1], F32, tag="wch1")
            nc.scalar.activation(wch1[:], psW[:], AF.Sigmoid, bias=wt['fc2_b'][:])
            nc.vector.tensor_scalar(wch1[:], wch1[:], 1.0, None, ALU.add)

            # Xe = X*(1+wch) in place (fp32r view), rows [2,70)
            Xe = A_[:].bitcast(F32R)
            nc.vector.scalar_tensor_tensor(Xe[:, 260:9100], A_[:, 260:9100],
                                           wch1[:], A_[:, 260:9100],
                                           op0=ALU.mult, op1=ALU.bypass)

            # =================== Phase 2: mlp + LN1 ===================
            # x in Cs (col j <-> flat j+CS_OFF... x range [260,9100): use CX_OFF=259)
            CX_OFF = 259
            x = Cs
            for tlo, tn in ntiles(260, 9100):
                ps = psum64.tile([64, 512], F32, tag="c64")
                nc.tensor.matmul(ps[0:64, 0:tn], wt['mlp_wT'][:], Xe[:, tlo:tlo + tn],
                                 start=True, stop=True)
                nc.vector.tensor_scalar(
                    x[0:64, tlo - CX_OFF:tlo - CX_OFF + tn].bitcast(F32R),
                    ps[0:64, 0:tn], wt['mlp_b'][:], None, ALU.add)

            ln1 = small.tile([64, 2], F32, tag="ln1")
            x3 = x[0:64, OWNLO - CX_OFF:OWNHI - CX_OFF] \
                .rearrange("p (r c) -> p r c", c=PITCH)[:, :, 1:129]
            nc.vector.reduce_sum(ln1[:, 0:1], x3, axis=AX.XY)
            sq1 = Bs[0:64, 0:8320].bitcast(F32R) \
                .rearrange("p (r c) -> p r c", c=PITCH)[:, :, 1:129]
            nc.scalar.activation(sq1, x3, AF.Square, accum_out=ln1[:, 1:2])
            nc.sync.dma_start(cc['l1_in'][:], ln1[:])
            allreduce('l1')
            ln1f = small.tile([64, 2], F32, tag="ln1f")
            nc.sync.dma_start(ln1f[:], cc['l1_out'][:])

            def ln_coeffs(lnf, tag):
                mu = small.tile([64, 1], F32, tag=tag + "mu")
                nc.vector.tensor_scalar(mu[:], lnf[:, 0:1], float(1.0 / NPIX), None, ALU.mult)
                musq = small.tile([64, 1], F32, tag=tag + "ms")
                nc.vector.tensor_tensor(musq[:], mu[:], mu[:], op=ALU.mult)
                var = small.tile([64, 1], F32, tag=tag + "va")
                nc.vector.scalar_tensor_tensor(var[:], lnf[:, 1:2], float(1.0 / NPIX),
                                               musq[:], op0=ALU.mult, op1=ALU.subtract)
                nc.vector.tensor_scalar(var[:], var[:], LN_EPS, None, ALU.add)
                sd = small.tile([64, 1], F32, tag=tag + "sd")
                nc.scalar.activation(sd[:], var[:], AF.Sqrt)
                inv = small.tile([64, 1], F32, tag=tag + "iv")
                nc.vector.reciprocal(inv[:], sd[:])
                nb = small.tile([64, 1], F32, tag=tag + "nb")
                nc.vector.tensor_tensor(nb[:], mu[:], inv[:], op=ALU.mult)
                nc.vector.tensor_scalar(nb[:], nb[:], -1.0, None, ALU.mult)
                return inv, nb

            inv1, nb1 = ln_coeffs(ln1f, "l1")
            # xn in A_ (in place over X/Xe; col j <-> flat j), fp32r, rows [2,70)
            xn = A_[:].bitcast(F32R)
            nc.scalar.activation(xn[0:64, 260:9100], x[0:64, 260 - CX_OFF:9100 - CX_OFF],
                                 AF.Identity, scale=inv1[:], bias=nb1[:])
            band_mul(A_, 260, 520, 260, pdim=64)
            band_mul(A_, 8840, 9100, 8840, pdim=64)
            padcol_memset(A_, 2, 70, base=260, pdim=64)

            # =================== Phase 3: MDTA ===================
            # qk1 in Bs (col j <-> flat j, range [390, 8970))
            qk1r = Bs[:].bitcast(F32R)
            for tlo, tn in ntiles(390, 8970):
                ps = psum.tile([128, 512], F32, tag="cps")
                nc.tensor.matmul(ps[:, 0:tn], wt['qk1_wT'][:], xn[0:64, tlo:tlo + tn],
                                 start=True, stop=True)
                nc.vector.tensor_scalar(qk1r[:, tlo:tlo + tn], ps[:, 0:tn],
                                        wt['qk1_b'][:], None, ALU.add)
            band_mul(Bs, 390, 520, 390)
            band_mul(Bs, 8840, 8970, 8840)
            padcol_memset(Bs, 3, 69, base=390)
            guard_memset(Bs, 389)
            guard_memset(Bs, 8970)

            # qk conv3x3 -> Fs (col j <-> flat j+FS_OFF=520), fp32r, padcols zeroed
            qkr = Fs[:].bitcast(F32R)
            wq2 = load_w3x3('qk2_wT')
            for tlo, tn in ntiles(OWNLO, OWNHI):
                ps = psum.tile([128, 512], F32, tag="cps")
                for t in range(9):
                    off = (t // 3 - 1) * PITCH + (t % 3 - 1)
                    nc.tensor.matmul(ps[:, 0:tn], wq2[:, t * 128:(t + 1) * 128],
                                     qk1r[:, tlo + off:tlo + off + tn],
                                     start=(t == 0), stop=(t == 8))
                nc.vector.tensor_scalar(qkr[:, tlo - FS_OFF:tlo - FS_OFF + tn],
                                        ps[:, 0:tn], wt['qk2_b'][:], None, ALU.add)
            padcol_memset(Fs, 4, 68, base=0)

            # transpose 65 blocks in place (qkT block i overwrites qk block i)
            for i in range(65):
                pst_t = pstr.tile([128, 128], F32R, tag="trps")
                nc.tensor.transpose(pst_t[:], qkr[:, i * 128:(i + 1) * 128], idr[:])
                nc.vector.tensor_copy(Fs[:, i * 128:(i + 1) * 128].bitcast(F32R), pst_t[:])
            psa = psA.tile([64, 64], F32, tag="Aps")
            for i in range(65):
                nc.tensor.matmul(psa[:], qkr[:, i * 128 + 64:(i + 1) * 128],
                                 qkr[:, i * 128:i * 128 + 64],
                                 start=(i == 0), stop=(i == 64))
            a_part = small.tile([64, 64], F32, tag="a_part")
            nc.vector.tensor_copy(a_part[:], psa[:])
            nc.sync.dma_start(cc['a_in'][:], a_part[:])
            allreduce('a')
            A_full = small.tile([64, 64], F32, tag="A_full")
            nc.sync.dma_start(A_full[:], cc['a_out'][:])

            # softmax over 4096 entries
            rmax = small.tile([64, 1], F32, tag="rmax")
            nc.vector.reduce_max(rmax[:], A_full[:], axis=AX.X)
            pt1 = pstr.tile([1, 64], F32, tag="tiny", name="pt1")
            nc.tensor.transpose(pt1[:], rmax[:], wt['ident'][0:64, 0:64])
            rmT = small.tile([1, 64], F32, tag="rmT")
            nc.vector.tensor_copy(rmT[:], pt1[:])
            gmax = small.tile([1, 1], F32, tag="gmax")
            nc.vector.reduce_max(gmax[:], rmT[:], axis=AX.X)
            nc.vector.tensor_scalar(gmax[:], gmax[:], -1.0, None, ALU.mult)
            ngmax = small.tile([64, 1], F32, tag="ngmax")
            nc.gpsimd.partition_broadcast(ngmax[:], gmax[:])
            E = small.tile([64, 64], F32, tag="E")
            rsum = small.tile([64, 1], F32, tag="rsum")
            nc.scalar.activation(E[:], A_full[:], AF.Exp, bias=ngmax[:], accum_out=rsum[:])
            pt2 = pstr.tile([1, 64], F32, tag="tiny", name="pt2")
            nc.tensor.transpose(pt2[:], rsum[:], wt['ident'][0:64, 0:64])
            rsT = small.tile([1, 64], F32, tag="rsT")
            nc.vector.tensor_copy(rsT[:], pt2[:])
            gsum = small.tile([1, 1], F32, tag="gsum")
            nc.vector.reduce_sum(gsum[:], rsT[:], axis=AX.X)
            nc.vector.reciprocal(gsum[:], gsum[:])
            isum = small.tile([64, 1], F32, tag="isum")
            nc.gpsimd.partition_broadcast(isum[:], gsum[:])
            A_sm = small.tile([64, 64], F32R, tag="A_sm")
            nc.scalar.activation(A_sm[:], E[:], AF.Identity, scale=isum[:])

            # v chain: v1 in Bs (col j <-> flat j, [260,9100)); v in Fs ([390,8970),
            # col j <-> flat j+FV_OFF=389)
            v1r = Bs[:].bitcast(F32R)
            for tlo, tn in ntiles(260, 9100):
                ps = psum64.tile([64, 512], F32, tag="c64")
                nc.tensor.matmul(ps[0:64, 0:tn], wt['v1_wT'][:], xn[0:64, tlo:tlo + tn],
                                 start=True, stop=True)
                nc.vector.tensor_scalar(v1r[0:64, tlo:tlo + tn], ps[0:64, 0:tn],
                                        wt['v1_b'][:], None, ALU.add)
            band_mul(Bs, 260, 520, 260, pdim=64)
            band_mul(Bs, 8840, 9100, 8840, pdim=64)
            padcol_memset(Bs, 2, 70, base=260, pdim=64)
            guard_memset(Bs, 259)
            guard_memset(Bs, 9100)

            FV_OFF = 389
            vr = Fs[:].bitcast(F32R)
            wv2 = load_w3x3('v2_wT')
            for tlo, tn in ntiles(390, 8970):
                ps = psum64.tile([64, 512], F32, tag="c64")
                for t in range(9):
                    off = (t // 3 - 1) * PITCH + (t % 3 - 1)
                    nc.tensor.matmul(ps[0:64, 0:tn], wv2[0:64, t * 64:(t + 1) * 64],
                                     v1r[0:64, tlo + off:tlo + off + tn],
                                     start=(t == 0), stop=(t == 8))
                nc.vector.tensor_scalar(vr[0:64, tlo - FV_OFF:tlo - FV_OFF + tn],
                                        ps[0:64, 0:tn], wt['v2_b'][:], None, ALU.add)

            # attn = A_sm^T v (in place over v), then x2 = w4 attn + b4 + x (in place)
            for tlo, tn in ntiles(390, 8970):
                ps = psum64.tile([64, 512], F32, tag="c64")
                nc.tensor.matmul(ps[0:64, 0:tn], A_sm[:],
                                 vr[0:64, tlo - FV_OFF:tlo - FV_OFF + tn],
                                 start=True, stop=True)
                nc.vector.tensor_copy(vr[0:64, tlo - FV_OFF:tlo - FV_OFF + tn],
                                      ps[0:64, 0:tn])
            x2 = Fs
            for tlo, tn in ntiles(390, 8970):
                ps = psum64.tile([64, 512], F32, tag="c64")
                nc.tensor.matmul(ps[0:64, 0:tn], wt['w4_wT'][:],
                                 vr[0:64, tlo - FV_OFF:tlo - FV_OFF + tn],
                                 start=True, stop=True)
                nc.vector.scalar_tensor_tensor(
                    x2[0:64, tlo - FV_OFF:tlo - FV_OFF + tn].bitcast(F32R),
                    ps[0:64, 0:tn], wt['w4_b'][:],
                    x[0:64, tlo - CX_OFF:tlo - CX_OFF + tn],
                    op0=ALU.add, op1=ALU.add)

            # LN2
            ln2 = small.tile([64, 2], F32, tag="ln2")
            x23 = x2[0:64, OWNLO - FV_OFF:OWNHI - FV_OFF] \
                .rearrange("p (r c) -> p r c", c=PITCH)[:, :, 1:129]
            nc.vector.reduce_sum(ln2[:, 0:1], x23, axis=AX.XY)
            sq2 = Bs[0:64, 0:8320].bitcast(F32R) \
                .rearrange("p (r c) -> p r c", c=PITCH)[:, :, 1:129]
            nc.scalar.activation(sq2, x23, AF.Square, accum_out=ln2[:, 1:2])
            nc.sync.dma_start(cc['l2_in'][:], ln2[:])
            allreduce('l2')
            ln2f = small.tile([64, 2], F32, tag="ln2f")
            nc.sync.dma_start(ln2f[:], cc['l2_out'][:])
            inv2, nb2 = ln_coeffs(ln2f, "l2")

            # xn2 in Bs (col j <-> flat j, rows [3,69)), fp32r
            xn2 = Bs[:].bitcast(F32R)
            nc.scalar.activation(xn2[0:64, 390:8970],
                                 x2[0:64, 390 - FV_OFF:8970 - FV_OFF],
                                 AF.Identity, scale=inv2[:], bias=nb2[:])
            band_mul(Bs, 390, 520, 390, pdim=64)
            band_mul(Bs, 8840, 8970, 8840, pdim=64)
            padcol_memset(Bs, 3, 69, base=390, pdim=64)

            # =================== Phase 4: GDFN (4 chunks of 16 rows) ===================
            HSZ = 18 * PITCH  # 2340
            OSZ = 16 * PITCH  # 2080
            # h tiles: h0..h2 in Cs at [0,2342),[2342,4684),[4684,7026); h3 in Gs [0,2342)
            # g tiles: g_lo in A_[:, 0:2080], g_hi in A_[:, 2080:4160]
            # out tile: Gs[64, 2342:4422]
            for ci in range(4):
                r_out0 = 4 + 16 * ci
                hbase = (r_out0 - 1) * PITCH
                hs = []
                for mo in range(4):
                    if mo < 3:
                        ht = Cs[:, mo * 2342:(mo + 1) * 2342]
                    else:
                        ht = Gs[:, 0:2342]
                    htr = ht.bitcast(F32R)
                    for tlo, tn in ntiles(0, HSZ):
                        ps = psum.tile([128, 512], F32, tag="cps")
                        nc.tensor.matmul(ps[:, 0:tn],
                                         wt['dd1_wT'][:, mo * 128:(mo + 1) * 128],
                                         xn2[0:64, hbase + tlo:hbase + tlo + tn],
                                         start=True, stop=True)
                        nc.vector.tensor_scalar(htr[:, 1 + tlo:1 + tlo + tn],
                                                ps[:, 0:tn],
                                                wt['dd1_b'][:, mo:mo + 1], None, ALU.add)
                    if ci == 0:
                        nc.vector.tensor_tensor(htr[:, 1:131], ht[:, 1:131],
                                                edge[:, 390:520], op=ALU.mult)
                    if ci == 3:
                        nc.vector.tensor_tensor(htr[:, 1 + 17 * PITCH:1 + HSZ],
                                                ht[:, 1 + 17 * PITCH:1 + HSZ],
                                                edge[:, 520:650], op=ALU.mult)
                    hap = htr[:, 1:1 + HSZ].rearrange("p (r c) -> p r c", c=PITCH)
                    zin18 = zr[:, 0:18].rearrange("p (r c) -> p r c", c=1)
                    nc.vector.tensor_copy(hap[:, :, 0:1], zin18)
                    nc.vector.tensor_copy(hap[:, :, 129:130], zin18)
                    nc.vector.tensor_copy(htr[:, 0:1], zr[:, 0:1])
                    nc.vector.tensor_copy(htr[:, 1 + HSZ:2 + HSZ], zr[:, 0:1])
                    hs.append(htr)

                # conv3x3 per (mo): d1 -> gelu into g tiles; d2 -> g = gelu*d2 (in place)
                for mo in range(4):
                    br, mb = mo // 2, mo % 2
                    gdst = A_[:, mb * 2080:(mb + 1) * 2080]
                    for kb in range(2):
                        t = dd2r[kb]
                        nc.sync.dma_start(
                            t[:],
                            ins['dd2_wT'][:, ((mo * 2 + kb) * 9) * 128:
                                          ((mo * 2 + kb) * 9 + 9) * 128])
                        nc.scalar.activation(t[:], t[:].bitcast(F32), AF.Copy)
                    for tlo, tn in ntiles(0, OSZ):
                        ps = psum.tile([128, 512], F32, tag="cps")
                        first = True
                        for kb in range(2):
                            src = hs[br * 2 + kb]
                            for t in range(9):
                                off = (t // 3 - 1) * PITCH + (t % 3 - 1)
                                base = 131 + tlo + off
                                nc.tensor.matmul(ps[:, 0:tn],
                                                 dd2r[kb][:, t * 128:(t + 1) * 128],
                                                 src[:, base:base + tn],
                                                 start=first, stop=(kb == 1 and t == 8))
                                first = False
                        if br == 0:
                            nc.scalar.activation(gdst[:, tlo:tlo + tn].bitcast(F32R),
                                                 ps[:, 0:tn],
                                                 AF.Gelu, bias=wt['dd2_b'][:, mo:mo + 1])
                        else:
                            nc.vector.scalar_tensor_tensor(
                                gdst[:, tlo:tlo + tn].bitcast(F32R), ps[:, 0:tn],
                                wt['dd2_b'][:, mo:mo + 1], gdst[:, tlo:tlo + tn],
                                op0=ALU.add, op1=ALU.mult)

                # wo + residual + DMA out
                outt = Gs[0:64, 2342:4422]
                obase = r_out0 * PITCH
                gr = A_[:].bitcast(F32R)
                for tlo, tn in ntiles(0, OSZ):
                    ps = psum64.tile([64, 512], F32, tag="c64")
                    for kb in range(2):
                        nc.tensor.matmul(ps[0:64, 0:tn],
                                         wt['wo_wT'][:, kb * 64:(kb + 1) * 64],
                                         gr[:, kb * 2080 + tlo:kb * 2080 + tlo + tn],
                                         start=(kb == 0), stop=(kb == 1))
                    nc.vector.scalar_tensor_tensor(
                        outt[:, tlo:tlo + tn].bitcast(F32R), ps[0:64, 0:tn],
                        wt['wo_b'][:],
                        x2[0:64, obase + tlo - FV_OFF:obase + tlo - FV_OFF + tn],
                        op0=ALU.add, op1=ALU.add)
                oap = outt[:, :].rearrange("p (r c) -> p r c", c=PITCH)[:, :, 1:129]
                nc.sync.dma_start(
                    out_t[:].rearrange("p (r c) -> p r c", c=W)[:, 16 * ci:16 * ci + 16, :],
                    oap)

    nc.compile()
    return nc


def _get_nc():
    if 'nc' not in _CACHE:
        _CACHE['nc'] = _build()
    return _CACHE['nc']


def make_in_maps(F_ct, F_mr, params):
    F_ct = np.asarray(F_ct, np.float32)
    F_mr = np.asarray(F_mr, np.float32)
    w = _pack_weights(params)
    in_maps = []
    for c in range(8):
        b, h = c // 2, c % 2
        m = dict(w)
        m['xct'] = _make_slab(F_ct, b, h)
        m['xmr'] = _make_slab(F_mr, b, h)
        m['edge'] = _make_edge(h)
        in_maps.append(m)
    return in_maps


def _get_exec():
    """Cached jitted 8-core executable: returns (fn, in_names, out_names)."""
    if 'exec' in _CACHE:
        return _CACHE['exec']
    import jax
    from jax.sharding import Mesh, PartitionSpec
    from jax.experimental.shard_map import shard_map
    from concourse import mybir
    from concourse.bass2jax import (install_neuronx_cc_hook, _bass_exec_p,
                                    partition_id_tensor)

    nc = _get_nc()
    install_neuronx_cc_hook()
    pname = nc.partition_id_tensor.name if nc.partition_id_tensor else None
    in_names, out_names, out_avals, zero_outs = [], [], [], []
    for alloc in nc.m.functions[0].allocations:
        if not isinstance(alloc, mybir.MemoryLocationSet):
            continue
        name = alloc.memorylocations[0].name
        if alloc.kind == 'ExternalInput':
            if name == pname:
                continue
            in_names.append(name)
        elif alloc.kind == 'ExternalOutput':
            out_names.append(name)
            shape = tuple(alloc.tensor_shape)
            dtype = mybir.dt.np(alloc.dtype)
            out_avals.append(jax.core.ShapedArray(shape, dtype))
            zero_outs.append(np.zeros(shape, dtype))
    n_params = len(in_names)
    all_names = in_names + out_names + ([pname] if pname else [])

    def _body(*args):
        operands = list(args)
        if pname:
            operands.append(partition_id_tensor())
        outs = _bass_exec_p.bind(
            *operands, out_avals=tuple(out_avals), in_names=tuple(all_names),
            out_names=tuple(out_names), lowering_input_output_aliases=(),
            sim_require_finite=True, sim_require_nnan=True, nc=nc)
        return tuple(outs)

    devices = jax.devices()[:8]
    mesh = Mesh(np.asarray(devices), ('core',))
    n_outs = len(out_names)
    sharded = jax.jit(
        shard_map(_body, mesh=mesh,
                  in_specs=(PartitionSpec('core'),) * (n_params + n_outs),
                  out_specs=(PartitionSpec('core'),) * n_outs,
                  check_rep=False),
        donate_argnums=tuple(range(n_params, n_params + n_outs)),
        keep_unused=True)
    _CACHE['exec'] = (sharded, in_names, out_names, out_avals, zero_outs)
    return _CACHE['exec']


def run_fast(in_maps):
    sharded, in_names, out_names, out_avals, zero_outs = _get_exec()
    concat_in = [np.concatenate([in_maps[c][nm] for c in range(8)], axis=0)
                 for nm in in_names]
    concat_zeros = [np.zeros((8 * z.shape[0], *z.shape[1:]), z.dtype)
                    for z in zero_outs]
    out_arrs = sharded(*concat_in, *concat_zeros)
    oi = out_names.index('out')
    full = np.asarray(out_arrs[oi]).reshape(8, *out_avals[oi].shape)
    out = np.zeros((B, C, H, W), np.float32)
    for c in range(8):
        b, h = c // 2, c % 2
        out[b, :, h * 64:(h + 1) * 64, :] = full[c].reshape(C, 64, W)
    return out


def run(F_ct, F_mr, params, trace=False):
    in_maps = make_in_maps(F_ct, F_mr, params)
    out = run_fast(in_maps)
    return out, None


def kernel(F_ct, F_mr, params):
    out, _ = run(F_ct, F_mr, params)
    return out
